# revision 6
# baseline (speedup 1.0000x reference)
"""Trainium2 Bass kernel for DKernelPredefinedSparseAttention — v2.

Problem: B=1, S=8192, H=16, D=128 attention; each 64-wide query block
attends to <=8 key blocks (kidx/kvalid block pattern + element-level
causal masking), softmax over gathered keys.

v2 design (vs v1 baseline):
  - All operands f16 (Q^T, K^T, V, probabilities): halves DMA traffic and
    makes every matmul 1 cycle/row at any width (fp32r needed >=256).
  - Whole schedule is host-specialized; k-blocks processed in pairs on the
    128 partitions; scores computed transposed S^T[kpos, q].
  - Masks applied via f16 matmul from a mask library, with consecutive
    q-block combos merged into single wider matmuls.
  - exp on ScalarE (scale folded), output f16.
  - Denominator l: DVE pre-adds the two slots of a group over their
    column intersection (halves PE ones-matmul columns), then ones-matmul.
  - No PSUM zero-init: one group-opening start=True per accumulator,
    pieces split at covered/uncovered boundaries for uniform PSUM state.
  - O^T/l evacuated once per chunk on DVE, stores on the GpSimd queue.
  - Big contiguous DMAs (2048-col f16 tiles), all loads prefetched.
  - Software pipelining: chunk c+1's QK/masks/exp issued before chunk c's
    l/PV close.
"""

import math
import numpy as np
import ml_dtypes

BS = 64          # sparse block size
CHUNK = 512      # q columns per PSUM accumulator bank
TS = 2048        # SBUF tile width for Q^T/K^T (columns)
NEG = -1.0e30    # host-side -inf for fallback math
MNEG = -30000.0  # f16-representable mask value (exp(scale*MNEG) == 0)


# ----------------------------------------------------------------------------
# host-side schedule construction
# ----------------------------------------------------------------------------

class _Tile:
    __slots__ = ("t", "q0", "q1", "width", "start_chunk", "mask_seqs",
                 "slot", "c0", "c1")

    def __init__(self, t, q0, q1):
        self.t = t
        self.q0 = q0                    # first q-block (inclusive)
        self.q1 = q1                    # last q-block (inclusive)
        self.width = (q1 - q0 + 1) * BS
        self.start_chunk = (q0 * BS) // CHUNK
        self.c0 = q0 * BS               # absolute first column
        self.c1 = (q1 + 1) * BS         # absolute end column
        self.mask_seqs = []             # (rel_block, [combo,...])
        self.slot = 0


class _Group:
    __slots__ = ("tiles", "inter")      # inter: (abs0, abs1) or None

    def __init__(self, tiles):
        self.tiles = tiles
        self.inter = None
        if len(tiles) == 2:
            a, b = tiles
            i0, i1 = max(a.c0, b.c0), min(a.c1, b.c1)
            if i1 > i0:
                self.inter = (i0, i1)


_COMBOS = [("D", "Z"), ("Z", "D"), ("D", "I"), ("I", "D"),
           ("I", "Z"), ("Z", "I"), ("I", "I")]
_COMBO_IDX = {c: i for i, c in enumerate(_COMBOS)}


def _build_consts(seqs):
    """Mask library lhsT [128,128] and sequence-combo rhs [128, 64*ncols].

    seqs: list of tuples of combo indices; entry k starts at column
    offset[k]*64 and spans len(seqs[k])*64 columns.
    """
    lib = np.zeros((128, 128), np.float32)
    for r in range(63):            # row r: top-diag mask column r
        lib[r, :64] = np.where(np.arange(64) <= r, 0.0, MNEG)
    for r in range(63, 126):       # row r: bottom-diag mask column r-63
        c = r - 63
        lib[r, 64:] = np.where(np.arange(64) <= c, 0.0, MNEG)
    lib[126, :64] = MNEG           # top-inf
    lib[127, 64:] = MNEG           # bottom-inf

    ncols = sum(len(s) for s in seqs)
    sel = np.zeros((128, 64 * max(ncols, 1)), np.float32)
    offs = []
    col = 0
    for seq in seqs:
        offs.append(col)
        for ci in seq:
            top, bot = _COMBOS[ci]
            for c in range(64):
                cc = col * 64 + c
                if top == "D":
                    if c < 63:
                        sel[c, cc] = 1.0
                elif top == "I":
                    sel[126, cc] = 1.0
                if bot == "D":
                    if c < 63:
                        sel[63 + c, cc] = 1.0
                elif bot == "I":
                    sel[127, cc] = 1.0
            col += 1
    return (lib.astype(np.float16), sel.astype(np.float16),
            offs)


def _contiguous_runs(mask):
    runs = []
    i, n = 0, len(mask)
    while i < n:
        if mask[i]:
            j = i
            while j + 1 < n and mask[j + 1]:
                j += 1
            runs.append((i, j))
            i = j + 1
        else:
            i += 1
    return runs


def _build_allow(kidx, kvalid, nb):
    allow = np.zeros((nb, nb), dtype=bool)
    kmax = kidx.shape[1]
    for i in range(nb):
        for jj in range(kmax):
            if kvalid[i, jj]:
                j = int(kidx[i, jj])
                if 0 <= j <= i:
                    allow[i, j] = True
    return allow


def _build_schedule(allow, nb, s):
    """Tiles, groups per start-chunk, and the mask-sequence table."""
    chunkb = CHUNK // BS
    nchunk = s // CHUNK
    tiles = []
    for t in range(nb // 2):
        j0, j1 = 2 * t, 2 * t + 1
        rows = allow[:, j0] | (allow[:, j1] if j1 < nb else False)
        for (a, b) in _contiguous_runs(list(rows)):
            if (b - a + 1) > chunkb:
                p = a
                while p <= b:
                    pe = min(b, (p // chunkb + 1) * chunkb - 1)
                    tiles.append(_Tile(t, p, pe))
                    p = pe + 1
            else:
                tiles.append(_Tile(t, a, b))

    # masks as merged sequences of consecutive q-block combos
    seq_table = {}
    seqs = []
    for T in tiles:
        combos = []
        for q in range(T.q0, T.q1 + 1):
            states = []
            for h in range(2):
                j = 2 * T.t + h
                if j >= nb or not allow[q, j]:
                    states.append("I")
                elif j == q:
                    states.append("D")
                else:
                    states.append("Z")
            combos.append(None if states == ["Z", "Z"]
                          else _COMBO_IDX[tuple(states)])
        # merge consecutive non-None entries into sequences
        i = 0
        n = len(combos)
        while i < n:
            if combos[i] is None:
                i += 1
                continue
            j = i
            while j + 1 < n and combos[j + 1] is not None:
                j += 1
            seq = tuple(combos[i:j + 1])
            if seq not in seq_table:
                seq_table[seq] = len(seqs)
                seqs.append(seq)
            T.mask_seqs.append((i, seq_table[seq]))
            i = j + 1

    # group tiles per start chunk (pairs, widest first)
    by_chunk = [[] for _ in range(nchunk)]
    for T in tiles:
        by_chunk[T.start_chunk].append(T)
    groups = [[] for _ in range(nchunk)]
    for c in range(nchunk):
        ts_sorted = sorted(by_chunk[c], key=lambda T: -T.width)
        for i in range(0, len(ts_sorted), 2):
            g = _Group(ts_sorted[i:i + 2])
            for slot, T in enumerate(g.tiles):
                T.slot = slot
            groups[c].append(g)
    return tiles, groups, seqs


def _split_chunks(a0, a1):
    """Split absolute col interval [a0,a1) at the CHUNK grid."""
    out = []
    p = a0
    while p < a1:
        pe = min(a1, (p // CHUNK + 1) * CHUNK)
        out.append((p // CHUNK, p, pe))
        p = pe
    return out


class _Coverage:
    """First-writer tracking for one PSUM accumulator (one chunk)."""

    def __init__(self, base):
        self.base = base
        self.cov = np.zeros(CHUNK, dtype=bool)

    def pieces(self, a0, a1):
        """Yield (a0, a1, start_flag) sub-runs; marks covered."""
        r0 = a0 - self.base
        r1 = a1 - self.base
        out = []
        p = r0
        while p < r1:
            st = not self.cov[p]
            j = p
            while j < r1 and (not self.cov[j]) == st:
                j += 1
            out.append((self.base + p, self.base + j, st))
            p = j
        self.cov[r0:r1] = True
        return out


# ----------------------------------------------------------------------------
# device program emission
# ----------------------------------------------------------------------------

def _emit_program(groups, seqs, seq_offs, s, hpc, n_cores, repeat=1):
    import concourse.bacc as bacc
    import concourse.tile as tile
    import concourse.mybir as mybir
    from contextlib import ExitStack

    f32 = mybir.dt.float32
    f16 = mybir.dt.float16
    Exp = mybir.ActivationFunctionType.Exp

    nchunk = s // CHUNK
    nt = math.ceil(s / TS)
    d = 128
    scale = 1.0 / math.sqrt(float(d))
    ncombo_cols = sum(len(sq) for sq in seqs)

    nc = bacc.Bacc("TRN2", debug=False, num_devices=n_cores)
    QT = nc.dram_tensor("QT", [hpc, d, s], f16, kind="ExternalInput").ap()
    KT = nc.dram_tensor("KT", [hpc, d, s], f16, kind="ExternalInput").ap()
    V = nc.dram_tensor("V", [hpc, 128, s // 128, d], f16,
                       kind="ExternalInput").ap()
    MASKLIB = nc.dram_tensor("MASKLIB", [128, 128], f16,
                             kind="ExternalInput").ap()
    COMBOS = nc.dram_tensor("COMBOS", [128, 64 * max(ncombo_cols, 1)], f16,
                            kind="ExternalInput").ap()
    ONES = nc.dram_tensor("ONES", [128, 1], f16, kind="ExternalInput").ap()
    OT = nc.dram_tensor("OT", [hpc, d, s], f32, kind="ExternalOutput").ap()
    LOUT = nc.dram_tensor("LOUT", [hpc, s], f32, kind="ExternalOutput").ap()

    with tile.TileContext(nc) as tc, ExitStack() as ctx:
        const_pool = ctx.enter_context(tc.tile_pool(name="consts", bufs=1))
        data_pool = ctx.enter_context(tc.tile_pool(name="data", bufs=1))
        pt_pool = ctx.enter_context(tc.tile_pool(name="pt", bufs=10))
        stg_pool = ctx.enter_context(tc.tile_pool(name="stg", bufs=3))
        ps_pool = ctx.enter_context(tc.tile_pool(name="ps", bufs=1,
                                                 space="PSUM"))

        masklib = const_pool.tile([128, 128], f16)
        combos = const_pool.tile([128, 64 * max(ncombo_cols, 1)], f16)
        ones = const_pool.tile([128, 1], f16)

        def load_consts():
            nc.sync.dma_start(masklib[:], MASKLIB)
            nc.sync.dma_start(combos[:], COMBOS)
            nc.sync.dma_start(ones[:], ONES)

        # persistent per-head tensor tiles, loaded just-in-time (2 chunks
        # ahead of first use) so stores interleave with loads cleanly
        kt_tiles = [[None] * nt for _ in range(hpc)]
        qt_tiles = [[None] * nt for _ in range(hpc)]
        v_tiles = [[None] * nt for _ in range(hpc)]
        chunks_per_ts = TS // CHUNK

        def load_tile_group(h, n, split=False):
            if kt_tiles[h][n] is not None:
                return
            w = min(TS, s - n * TS)
            kt = data_pool.tile([128, w], f16, name=f"kt{h}_{n}")
            qt = data_pool.tile([128, w], f16, name=f"qt{h}_{n}")
            a0 = (n * TS) // 128
            na = w // 128
            vt = data_pool.tile([128, na, 128], f16, name=f"vt{h}_{n}")
            if split and w > 512:
                # first slivers ASAP so chunk 0 can start early
                nc.sync.dma_start(kt[:, :512], KT[h][:, n * TS:n * TS + 512])
                nc.sync.dma_start(qt[:, :512], QT[h][:, n * TS:n * TS + 512])
                nc.sync.dma_start(vt[:, :4, :], V[h][:, a0:a0 + 4, :])
                nc.sync.dma_start(kt[:, 512:],
                                  KT[h][:, n * TS + 512:n * TS + w])
                nc.sync.dma_start(qt[:, 512:],
                                  QT[h][:, n * TS + 512:n * TS + w])
                nc.sync.dma_start(vt[:, 4:, :], V[h][:, a0 + 4:a0 + na, :])
            else:
                nc.sync.dma_start(kt[:], KT[h][:, n * TS:n * TS + w])
                nc.sync.dma_start(qt[:], QT[h][:, n * TS:n * TS + w])
                nc.sync.dma_start(vt[:], V[h][:, a0:a0 + na, :])
            kt_tiles[h][n] = kt
            qt_tiles[h][n] = qt
            v_tiles[h][n] = vt

        # loads_at[gc]: tile-groups to load when global chunk gc begins
        loads_at = [[] for _ in range(hpc * nchunk)]
        for h in range(hpc):
            for n in range(nt):
                gc = max(0, h * nchunk + n * chunks_per_ts - 2)
                loads_at[gc].append((h, n))
        load_tile_group(0, 0)   # first group immediately
        load_consts()

        for rep in range(repeat):
          for h in range(hpc):
            # pgrp/psums SBUF tiles and psum tiles per group, keyed by id
            pgrps = {}
            padds = {}
            ot_ps = [None] * nchunk
            l_ps = [None] * nchunk
            l_stages = {}
            # close-lists: (kind, ...) per chunk
            pv_close = [[] for _ in range(nchunk)]
            l_close = [[] for _ in range(nchunk)]

            # ---- plan l/PV segments on host ------------------------------
            for c in range(nchunk):
                for g in groups[c]:
                    for T in g.tiles:
                        for (ch, a0, a1) in _split_chunks(T.c0, T.c1):
                            pv_close[ch].append((g, T, a0, a1))
                    # l pieces: non-intersect parts from pgrp slots,
                    # intersect from the pre-added buffer
                    if g.inter is not None:
                        i0, i1 = g.inter
                        for T in g.tiles:
                            if T.c0 < i0:
                                for (ch, a0, a1) in _split_chunks(T.c0, i0):
                                    l_close[ch].append((g, T, a0, a1))
                            if T.c1 > i1:
                                for (ch, a0, a1) in _split_chunks(i1, T.c1):
                                    l_close[ch].append((g, T, a0, a1))
                        for (ch, a0, a1) in _split_chunks(i0, i1):
                            l_close[ch].append((g, None, a0, a1))
                    else:
                        for T in g.tiles:
                            for (ch, a0, a1) in _split_chunks(T.c0, T.c1):
                                l_close[ch].append((g, T, a0, a1))

            def emit_groups(c):
                for (hh, n) in loads_at[h * nchunk + c]:
                    load_tile_group(hh, n)
                for gi, g in enumerate(groups[c]):
                    nslot = len(g.tiles)
                    sg = ps_pool.tile([128, 2, CHUNK], f32, tag="sg", bufs=2,
                                      name=f"sg{h}_{c}_{gi}")
                    pgrp = pt_pool.tile([128, 2, CHUNK], f16, tag="pg",
                                        name=f"pg{h}_{c}_{gi}")
                    pgrps[id(g)] = pgrp
                    # QK (pieces split at the TS grid)
                    for T in g.tiles:
                        qkp = _split_ts(T.c0, T.c1)
                        kt = kt_tiles[h][(T.t * 128) // TS]
                        krel = (T.t * 128) % TS
                        for pi, (p0, p1) in enumerate(qkp):
                            n = p0 // TS
                            rel = p0 - T.c0
                            nc.tensor.matmul(
                                sg[:, T.slot, rel:rel + (p1 - p0)],
                                kt[:, krel:krel + 128],
                                qt_tiles[h][n][:, p0 - n * TS:p1 - n * TS],
                                start=(pi == 0),
                                stop=(pi == len(qkp) - 1 and
                                      not T.mask_seqs),
                                skip_group_check=True)
                    # masks
                    for T in g.tiles:
                        nm = len(T.mask_seqs)
                        for mi, (rb, si) in enumerate(T.mask_seqs):
                            off = seq_offs[si]
                            w = 64 * len(seqs[si])
                            nc.tensor.matmul(
                                sg[:, T.slot, rb * BS:rb * BS + w],
                                masklib[:],
                                combos[:, off * 64:off * 64 + w],
                                start=False, stop=(mi == nm - 1),
                                skip_group_check=True)
                    # exp
                    if nslot == 2 and g.tiles[0].width == g.tiles[1].width:
                        w = g.tiles[0].width
                        nc.scalar.activation(pgrp[:, :, :w], sg[:, :, :w],
                                             Exp, scale=scale)
                    else:
                        for T in g.tiles:
                            nc.scalar.activation(
                                pgrp[:, T.slot, :T.width],
                                sg[:, T.slot, :T.width], Exp, scale=scale)
                    # DVE pre-add over the intersection
                    if g.inter is not None:
                        i0, i1 = g.inter
                        a, b = g.tiles
                        padd = pt_pool.tile([128, CHUNK], f16, tag="pa",
                                            name=f"pa{h}_{c}_{gi}")
                        padds[id(g)] = padd
                        nc.vector.tensor_add(
                            padd[:, :i1 - i0],
                            pgrp[:, 0, i0 - a.c0:i1 - a.c0],
                            pgrp[:, 1, i0 - b.c0:i1 - b.c0])

            LB = 4          # l chunks batched per PSUM bank

            def emit_close(c):
                if not pv_close[c] and not l_close[c]:
                    return
                if ot_ps[c] is None:
                    ot_ps[c] = ps_pool.tile([128, CHUNK], f32, tag="ot",
                                            bufs=2, name=f"ot{h}_{c}")
                cb = c // LB
                if l_ps[c] is None:
                    l_ps[c] = ps_pool.tile([1, CHUNK], f32, tag="l",
                                           bufs=2, name=f"l{h}_{c}")
                base = c * CHUNK
                # PV first (per-tile V stationary), then l (ones stationary).
                # PSUM semantics: the FIRST matmul per accumulator opens the
                # group (start=True marks the whole bank pending-zero); later
                # matmuls use start=False — pending bytes overwrite, written
                # bytes accumulate — but each matmul must touch a UNIFORM
                # state, so split pieces at covered/uncovered boundaries.
                cov_ot = _Coverage(base)
                plan_pv = []
                for (g, T, a0, a1) in pv_close[c]:
                    for (p0, p1, _) in cov_ot.pieces(a0, a1):
                        plan_pv.append((g, T, p0, p1, not plan_pv))
                cov_l = _Coverage(base)
                plan_l = []
                for (g, T, a0, a1) in l_close[c]:
                    for (p0, p1, _) in cov_l.pieces(a0, a1):
                        plan_l.append((g, T, p0, p1, not plan_l))
                for i, (g, T, p0, p1, st) in enumerate(plan_pv):
                    vt = v_tiles[h][(T.t * 128) // TS]
                    vrel = (T.t * 128 % TS) // 128
                    nc.tensor.matmul(
                        ot_ps[c][:, p0 - base:p1 - base],
                        vt[:, vrel, :],
                        pgrps[id(g)][:, T.slot, p0 - T.c0:p1 - T.c0],
                        start=st, stop=(i == len(plan_pv) - 1),
                        skip_group_check=True)
                for i, (g, T, p0, p1, st) in enumerate(plan_l):
                    if T is None:
                        src = padds[id(g)][:, p0 - g.inter[0]:p1 - g.inter[0]]
                    else:
                        src = pgrps[id(g)][:, T.slot, p0 - T.c0:p1 - T.c0]
                    nc.tensor.matmul(
                        l_ps[c][:, p0 - base:p1 - base],
                        ones[:], src,
                        start=st, stop=(i == len(plan_l) - 1),
                        skip_group_check=True)
                # evacuate PSUM via DVE; store on the idle GpSimd SWDGE queue
                ot_stage = stg_pool.tile([128, CHUNK], f32, tag="ots",
                                         bufs=4, name=f"ots{h}_{c}")
                nc.vector.tensor_copy(ot_stage[:], ot_ps[c][:])
                nc.gpsimd.dma_start(OT[h][:, base:base + CHUNK], ot_stage[:])
                if c % LB == 0:
                    l_stages[cb] = stg_pool.tile([1, LB * CHUNK], f32,
                                                 tag="ls", bufs=2,
                                                 name=f"ls{h}_{cb}")
                nc.vector.tensor_copy(
                    l_stages[cb][:, (c % LB) * CHUNK:(c % LB + 1) * CHUNK],
                    l_ps[c][:])
                if c % LB == LB - 1 or c == nchunk - 1:
                    nrow = c % LB + 1
                    lbase = cb * LB * CHUNK
                    nc.gpsimd.dma_start(
                        LOUT[h][lbase:lbase + nrow * CHUNK].rearrange(
                            "(r c) -> r c", r=1),
                        l_stages[cb][:, :nrow * CHUNK])

            # software-pipelined emission: groups(c+1) before close(c)
            emit_groups(0)
            for c in range(1, nchunk):
                emit_groups(c)
                emit_close(c - 1)
            emit_close(nchunk - 1)

    nc.compile()
    return nc


def _split_ts(c0, c1):
    out = []
    p = c0
    while p < c1:
        pe = min(c1, (p // TS + 1) * TS)
        out.append((p, pe))
        p = pe
    return out


# ----------------------------------------------------------------------------
# host entry point
# ----------------------------------------------------------------------------

def _host_fallback(out, q, k, v, kidx, kvalid, blocks):
    b, s, h, d = q.shape
    nb = s // BS
    kmax = kidx.shape[1]
    kb = k.reshape(nb, BS, h, d)
    vb = v.reshape(nb, BS, h, d)
    scale = 1.0 / math.sqrt(d)
    for i in blocks:
        qb = q[0, i * BS:(i + 1) * BS]
        kg = kb[kidx[i]]
        vg = vb[kidx[i]]
        scores = np.einsum("ahd,kchd->hakc", qb, kg) * scale
        qpos = i * BS + np.arange(BS)
        kpos = kidx[i][:, None] * BS + np.arange(BS)[None, :]
        ok = (qpos[:, None, None] >= kpos[None, :, :]) & \
            kvalid[i][None, :, None]
        scores = np.where(ok[None], scores, NEG)
        sc = scores.reshape(h, BS, kmax * BS)
        sc = sc - sc.max(axis=-1, keepdims=True)
        e = np.exp(sc)
        p = e / e.sum(axis=-1, keepdims=True)
        o = np.einsum("hak,khd->ahd", p, vg.reshape(kmax * BS, h, d))
        out[0, i * BS:(i + 1) * BS] = o


def _prepare(q, k, v, kidx, kvalid, n_cores):
    b, s, h, d = q.shape
    assert b == 1 and d == 128 and s % CHUNK == 0
    hpc = h // n_cores
    nb = s // BS

    kidx = np.asarray(kidx, dtype=np.int32)
    kvalid = np.asarray(kvalid, dtype=bool)

    allow = _build_allow(kidx, kvalid, nb)
    fallback = [i for i in range(nb) if not allow[i].any()]

    tiles, groups, seqs = _build_schedule(allow, nb, s)
    masklib, combos, seq_offs = _build_consts(seqs)
    nc = _emit_program(groups, seqs, seq_offs, s, hpc, n_cores)

    ones = np.ones((128, 1), np.float16)
    in_maps = []
    for c in range(n_cores):
        hs = slice(c * hpc, (c + 1) * hpc)
        qh = q[0, :, hs, :]                                  # [s, hpc, d]
        kh = k[0, :, hs, :]
        vh = v[0, :, hs, :]
        # V layout [hpc, p=128, a=s/128, d]: v16[h, p, a, :] = v[a*128+p]
        v16 = np.ascontiguousarray(
            vh.transpose(1, 0, 2).reshape(hpc, s // 128, 128, d)
            .transpose(0, 2, 1, 3)).astype(np.float16)
        in_maps.append({
            "QT": np.ascontiguousarray(qh.transpose(1, 2, 0)).astype(
                np.float16),
            "KT": np.ascontiguousarray(kh.transpose(1, 2, 0)).astype(
                np.float16),
            "V": v16,
            "MASKLIB": masklib,
            "COMBOS": combos,
            "ONES": ones,
        })
    return nc, in_maps, fallback


def _postprocess(results, q, k, v, kidx, kvalid, fallback, n_cores):
    b, s, h, d = q.shape
    hpc = h // n_cores
    out = np.empty((b, s, h, d), dtype=np.float32)
    for c in range(n_cores):
        for hh in range(hpc):
            ot = results[c]["OT"][hh]                        # [d, s]
            l = results[c]["LOUT"][hh]                       # [s]
            out[0, :, c * hpc + hh, :] = (ot / l[None, :]).T
    if fallback:
        _host_fallback(out, q, k, v, np.asarray(kidx, np.int32),
                       np.asarray(kvalid, bool), fallback)
    return out


def _attention_forward(q, k, v, kidx, kvalid, n_cores):
    from concourse import bass_utils

    nc, in_maps, fallback = _prepare(q, k, v, kidx, kvalid, n_cores)
    res = bass_utils.run_bass_kernel_spmd(
        nc, in_maps, core_ids=list(range(n_cores)))
    out = _postprocess(res.results, q, k, v, kidx, kvalid, fallback, n_cores)
    if res.exec_time_ns is not None:
        print(f"HW exec time: {res.exec_time_ns} ns")
    return out


def kernel(q, k, v, kidx, kvalid):
    return _attention_forward(
        np.asarray(q, dtype=np.float32), np.asarray(k, dtype=np.float32),
        np.asarray(v, dtype=np.float32), np.asarray(kidx),
        np.asarray(kvalid), n_cores=8)
